# revision 34
# baseline (speedup 1.0000x reference)
"""Trainium2 Bass kernel for a dense pre-norm transformer block (B=2, T=2048,
D=2048, H=16, DH=128, FG=5461, SwiGLU MLP, RoPE, causal attention).

Sharding: tensor-parallel attention over heads (2 heads/core on 8 cores), one
AllToAll per head-slot to reshard to 512 query columns per core, then fully
local proj + MLP per core (weights replicated, streamed from HBM).

v2: all matmuls in bf16 (fp32 PSUM accumulation), qkv kept SBUF-resident,
softmax denominators via DVE accumulate + one ones-matmul per q-block,
causal diagonal column trim, proj split into even/odd head halves so the
first half overlaps the second AllToAll.

v3: attention pipeline restructure. Score tiles processed as kt-PAIRS: two
score matmuls land in one 2-bank PSUM tile, one paired mask-add (DVE) and
one paired exp (ACT) cover both halves (a diagonal pair's masked half exps
to ~0, so only the last pair of each q-block needs an offset slice). The
next pair's score matmuls are issued ahead of the exp-dependent o-matmuls
so the PE never waits on ACT. The two batch streams of each head slot are
interleaved at q-block granularity so each stream's exp/finalize latency is
covered by the other's matmul work. Softmax numerator sums run as two
parallel bf16 accumulator chains on DVE (2x 16-bit rate, half the serial
depth) and feed the ones-matmul denominator reduction directly.
v-transposes are staged causally (the 4 k-tiles each q-block's diagonal
introduces, pair-packed into one PSUM bank + one paired ACT copy). Rope
raw copies moved to ACT; each rope block (rot matmul + DVE) deferred by
one qkv chain so it never head-of-line-blocks the tensor queue while ACT
drains the squares backlog. First x row-block split across three DMA
queues (sync/scalar/gpsimd); cos/sin on sync so qkv weights stream
back-to-back on scalar. PE warmup matmuls during the startup DMA wait
keep the array clock warm (first qkv chains run at 264 ns instead of the
cold 426 ns). proj-A's first weight chunks and its otf inputs prefetch on
the sync queue during h1's attention (the otf loads wait on the h0
collective semaphore, which is free: A2A(h1) is CC-serialized behind
A2A(h0) anyway); xres streams on sync inside phase 4 so it doesn't block
the proj weight stream on scalar.

Note: the mere presence of a collective in the NEFF pins the PE clock at
~1.95 GHz (263 ns per 128x512 bf16 matmul vs 216 ns without, HAM state 31
vs 0) for the entire kernel; the AllToAll reshard is structurally required
(any collective-free alternative does >= 3x redundant qkv compute), so the
MLP/proj phases run at that clock, 99%+ PE-busy. fp8 DoubleRow (a true 2x
at the instruction level, validated on HW) is numerically dead here: with
K=2048 contractions even a single fp8 e4m3 stage costs ~2.8e-2 max-rel
error against the 2e-2 gate (bf16 total: 3.4e-3).

All on-device activations are kept transposed ([feature, row]) so every
matmul is lhsT(=weight tile).T @ rhs(=activationT tile) with the contraction
dim on SBUF partitions.
"""

import numpy as np
import ml_dtypes

import concourse.bass as bass
import concourse.mybir as mybir
import concourse.tile as tile
from concourse import bacc
from concourse.bass_utils import run_bass_kernel_spmd

# Problem constants
B, T, D = 2, 2048, 2048
H, DH = 16, 128
FG = 5461
EPS = 1e-5
ROPE_BASE = 10000.0

P = 128
NCORES = 8
R = B * T                    # 4096 rows total
RB = 512                     # rows per block / per-core q-cols
NRB = R // RB                # 8 row blocks
DT = D // P                  # 16 d-tiles
HPC = H // NCORES            # 2 heads per core
NF = 3 * HPC                 # 6 feature tiles per core in qkv (q0,q1,k0,k1,v0,v1)
FGP = 5504                   # FG padded to 43*128
FGT = FGP // P               # 43 fg tiles
KTB = T // P                 # 16 k-tiles per batch
QBB = T // RB                # 4 q-blocks per batch
NEG = -1.0e30
SCALE = 1.0 / np.sqrt(DH)

NPBF = ml_dtypes.bfloat16

F32 = mybir.dt.float32
BF16 = mybir.dt.bfloat16
EXP = mybir.ActivationFunctionType.Exp
SQUARE = mybir.ActivationFunctionType.Square
SILU = mybir.ActivationFunctionType.Silu
MULT = mybir.AluOpType.mult
ADD = mybir.AluOpType.add


def _build_program():
    nc = bacc.Bacc("TRN2", target_bir_lowering=False, debug=False, num_devices=NCORES)

    # ---- per-core external inputs ----
    xT = nc.dram_tensor("xT", [D, R], BF16, kind="ExternalInput")
    xres = nc.dram_tensor("xres", [DT, P, RB], F32, kind="ExternalInput")
    wqkv = nc.dram_tensor("wqkv", [P, NF, DT, P], BF16, kind="ExternalInput")
    wproj = nc.dram_tensor("wproj", [P, DT, DT, P], BF16, kind="ExternalInput")
    wgate = nc.dram_tensor("wgate", [P, FGT, DT, P], BF16, kind="ExternalInput")
    wval = nc.dram_tensor("wval", [P, FGT, DT, P], BF16, kind="ExternalInput")
    wmlp = nc.dram_tensor("wmlp", [P, DT, FGT, P], BF16, kind="ExternalInput")
    cosT = nc.dram_tensor("cosT", [P, T], BF16, kind="ExternalInput")
    sinT = nc.dram_tensor("sinT", [P, T], BF16, kind="ExternalInput")
    rotPT = nc.dram_tensor("rotPT", [P, P], BF16, kind="ExternalInput")
    masks = nc.dram_tensor("masks", [P, QBB, RB], BF16, kind="ExternalInput")
    ones_in = nc.dram_tensor("ones_in", [P, P], BF16, kind="ExternalInput")
    ident_in = nc.dram_tensor("ident_in", [P, P], BF16, kind="ExternalInput")

    outT = nc.dram_tensor("outT", [DT, P, RB], F32, kind="ExternalOutput")

    # ---- internal DRAM scratch (collectives must use internal DRAM) ----
    a2a_in = [nc.dram_tensor(f"a2a_in{h}", [NCORES, P, RB], BF16) for h in range(HPC)]
    a2a_out = [nc.dram_tensor(f"a2a_out{h}", [NCORES, P, RB], BF16) for h in range(HPC)]

    with tile.TileContext(nc) as tc:
        with tc.tile_pool(name="const", bufs=1) as cpool:
            rot_t = cpool.tile([P, P], BF16)
            ones_t = cpool.tile([P, P], BF16)
            id_t = cpool.tile([P, P], BF16)
            # prefetched phase-4A inputs: loaded on the (idle) sync queue
            # during h1's attention, so proj-A starts without input stalls.
            # The otf_A loads wait on the h0 collective semaphore, which
            # delays h1's a2a stores behind them -- harmless, because the CC
            # queue serializes A2A(h1) behind A2A(h0) completion anyway.
            wp_pre = cpool.tile([P, 4, 8, P], BF16)
            otf_A = cpool.tile([P, NCORES, RB], BF16)

            with tc.tile_pool(name="attc", bufs=1) as apool:
                cos_t = apool.tile([P, T], BF16)
                sin_t = apool.tile([P, T], BF16)
                mask_t = apool.tile([P, QBB, RB], BF16)

                # qkv for this core's 2 heads over all rows, SBUF-resident
                qkv_sb = apool.tile([P, NF, R], BF16)

                # ============ Phase 1: qkv on raw x + rope, rms at output ======
                with (
                    tc.tile_pool(name="p1w", bufs=1) as p1w,
                    tc.tile_pool(name="p1x", bufs=2) as p1x,
                    tc.tile_pool(name="p1t", bufs=3) as p1t,
                    tc.tile_pool(name="p1ps_ss", bufs=1, space="PSUM") as p1ps_ss,
                    tc.tile_pool(name="p1ps_mm", bufs=4, space="PSUM") as p1ps_mm,
                    tc.tile_pool(name="p1ps_rot", bufs=2, space="PSUM") as p1ps_rot,
                ):
                    wq_t = [p1w.tile([P, DT, P], BF16, tag=f"wq{f}", name=f"wq{f}")
                            for f in range(NF)]
                    xTr = xT.rearrange("(dt p) r -> p dt r", p=P)
                    # first weight tile + first x block first, so the first
                    # matmul chain starts as soon as possible; bulky constants
                    # (cos/sin/masks) come after the startup-critical loads
                    nc.scalar.dma_start(rot_t[:], rotPT[:, :])
                    nc.scalar.dma_start(wq_t[0][:], wqkv[:, 0, :, :])
                    xb0 = p1x.tile([P, DT, RB], BF16, tag="xblk")
                    for dt in range(DT):
                        eng = (nc.sync if dt % 2 == 0
                               else (nc.scalar if dt % 4 == 1 else nc.gpsimd))
                        eng.dma_start(xb0[:, dt], xTr[:, dt, 0:RB])
                    # PE warmup during the startup DMA wait: junk matmuls on
                    # the (tiny, already-arrived) rot tile keep the array's
                    # clock state warm so the first qkv chains run at speed
                    warm_ps = p1ps_ss.tile([P, P], F32, tag="warm")
                    for _ in range(48):
                        nc.tensor.matmul(warm_ps[:], rot_t[:], rot_t[:],
                                         start=True, stop=True)
                    nc.scalar.dma_start(ones_t[:], ones_in[:, :])
                    nc.scalar.dma_start(cos_t[:], cosT[:, :])
                    nc.scalar.dma_start(sin_t[:], sinT[:, :])
                    for f in range(1, NF):
                        nc.scalar.dma_start(wq_t[f][:], wqkv[:, f, :, :])
                    nc.scalar.dma_start(id_t[:], ident_in[:, :])
                    nc.scalar.dma_start(mask_t[:], masks[:, :, :])

                    for rb in range(NRB):
                        t0 = (rb % QBB) * RB
                        if rb == 0:
                            xb = xb0
                        else:
                            xb = p1x.tile([P, DT, RB], BF16, tag="xblk")
                            for dt in range(DT):
                                nc.sync.dma_start(
                                    xb[:, dt],
                                    xTr[:, dt, rb * RB : (rb + 1) * RB],
                                )
                        # rms chain (independent of qkv matmuls; scale at output)
                        # squares on ACT, partial-sum accumulation on Pool
                        sacc = p1t.tile([P, RB], F32, tag="sacc")
                        first_sq = None
                        for dt in range(DT):
                            sq = p1t.tile([P, RB], F32, tag="sq")
                            nc.scalar.activation(sq[:], xb[:, dt], SQUARE)
                            if dt == 0:
                                first_sq = sq
                            elif dt == 1:
                                nc.vector.tensor_tensor(sacc[:], first_sq[:], sq[:], ADD)
                            else:
                                nc.vector.tensor_tensor(sacc[:], sacc[:], sq[:], ADD)
                        sacc_bf = p1t.tile([P, RB], BF16, tag="saccbf")
                        nc.vector.tensor_copy(sacc_bf[:], sacc[:])
                        ss_ps = p1ps_ss.tile([P, RB], F32, tag="ss")
                        nc.tensor.matmul(ss_ps[:], ones_t[:], sacc_bf[:],
                                         start=True, stop=True)
                        ms = p1t.tile([P, RB], F32, tag="ms")
                        nc.vector.tensor_scalar(ms[:], ss_ps[:], 1.0 / D, EPS, MULT, ADD)
                        nc.scalar.sqrt(ms[:], ms[:])
                        rms = p1t.tile([P, RB], F32, tag="rms")
                        nc.vector.reciprocal_approx_fast(rms[:], ms[:])
                        # cos/sin pre-scaled by rms for this row block
                        csr = p1t.tile([P, RB], F32, tag="csr")
                        nc.vector.tensor_tensor(csr[:], cos_t[:, t0 : t0 + RB], rms[:], MULT)
                        ssr = p1t.tile([P, RB], F32, tag="ssr")
                        nc.vector.tensor_tensor(ssr[:], sin_t[:, t0 : t0 + RB], rms[:], MULT)
                        # qkv matmuls on RAW x; f: 0,1=q; 2,3=k; 4,5=v.
                        # Each rope block (rot matmul + DVE) is deferred by
                        # one chain: the rot matmul waits on an ACT raw-copy
                        # that sits behind the squares backlog, and emitting
                        # it immediately would head-of-line-block the next
                        # qkv chain on the tensor queue.
                        def emit_rope(pend):
                            ps_p, raw_p, dst_p = pend
                            rps = p1ps_rot.tile([P, RB], F32, tag="rotps")
                            nc.tensor.matmul(rps[:], rot_t[:], raw_p[:],
                                             start=True, stop=True)
                            m1 = p1t.tile([P, RB], F32, tag="m1")
                            nc.vector.tensor_tensor(m1[:], ps_p[:], csr[:], MULT)
                            m2 = p1t.tile([P, RB], F32, tag="m2")
                            nc.vector.tensor_tensor(m2[:], rps[:], ssr[:], MULT)
                            nc.vector.tensor_tensor(dst_p, m1[:], m2[:], ADD)

                        pend_rope = None
                        for f in range(NF):
                            ps = p1ps_mm.tile([P, RB], F32, tag="qkvps")
                            for dt in range(DT):
                                nc.tensor.matmul(
                                    ps[:], wq_t[f][:, dt], xb[:, dt],
                                    start=(dt == 0), stop=(dt == DT - 1),
                                )
                            if pend_rope is not None:
                                emit_rope(pend_rope)
                                pend_rope = None
                            dst = qkv_sb[:, f, rb * RB : (rb + 1) * RB]
                            if f < 2 * HPC:
                                raw = p1t.tile([P, RB], BF16, tag="raw")
                                nc.scalar.copy(raw[:], ps[:])
                                pend_rope = (ps, raw, dst)
                            else:
                                nc.vector.tensor_tensor(dst, ps[:], rms[:], MULT)

                # ============ Phase 2: attention, h outer (A2A per head) ========
                with (
                    tc.tile_pool(name="p2kv", bufs=2) as p2kv,
                    tc.tile_pool(name="p2a", bufs=6) as p2a,
                    tc.tile_pool(name="p2t", bufs=3) as p2t,
                    tc.tile_pool(name="p2ps_tp", bufs=1, space="PSUM") as p2ps_tp,
                    tc.tile_pool(name="p2ps_s", bufs=2, space="PSUM") as p2ps_s,
                    tc.tile_pool(name="p2ps_o", bufs=2, space="PSUM") as p2ps_o,
                    tc.tile_pool(name="p2ps_l", bufs=1, space="PSUM") as p2ps_l,
                ):
                    def emit_spair(st, kp):
                        """Issue the two score matmuls for kt-pair kp."""
                        qb = st["qb"]
                        kt0 = 2 * kp
                        di0 = kt0 - 4 * qb
                        off0 = di0 * P if di0 > 0 else 0
                        off1 = (di0 + 1) * P if di0 + 1 > 0 else 0
                        spair = p2ps_s.tile([P, 2, RB], F32, tag="sps")
                        nc.tensor.matmul(
                            spair[:, 0, off0:],
                            st["kT"][:, kt0 * P : (kt0 + 1) * P],
                            st["qTs"][:, off0:], start=True, stop=True)
                        nc.tensor.matmul(
                            spair[:, 1, off1:],
                            st["kT"][:, (kt0 + 1) * P : (kt0 + 2) * P],
                            st["qTs"][:, off1:], start=True, stop=True)
                        return spair, di0, off0, off1

                    def emit_qb(st):
                        """Emit one q-block for a stream, with the next
                        kt-pair's score matmuls issued ahead of each o-chain
                        step so the PE never waits on exp."""
                        h, b, qb = st["h"], st["b"], st["qb"]
                        vT = st["vT"]
                        v_rm = st["v_rm"]
                        # v-transposes staged causally: the 4 k-tiles this
                        # q-block's diagonal introduces
                        for kt0 in range(4 * qb, 4 * qb + 4, 2):
                            tps = p2ps_tp.tile([P, 2, P], BF16, tag="vtp")
                            nc.tensor.transpose(
                                tps[:, 0], vT[:, kt0 * P : (kt0 + 1) * P],
                                id_t[:])
                            nc.tensor.transpose(
                                tps[:, 1], vT[:, (kt0 + 1) * P : (kt0 + 2) * P],
                                id_t[:])
                            nc.scalar.copy(v_rm[:, kt0 : kt0 + 2, :], tps[:])
                        nkt = 4 * qb + 4
                        np_ = nkt // 2
                        o_ps = p2ps_o.tile([P, RB], F32, tag="ops")
                        # two parallel bf16 numerator-sum chains (even/odd
                        # halves of each kt-pair) halve the serial depth; a
                        # diagonal pair's masked half is exp(NEG)~0, so odd
                        # halves accumulate full pair-width safely
                        laccE = p2t.tile([P, RB], BF16, tag="laccE")
                        laccO = p2t.tile([P, RB], BF16, tag="laccO")
                        pend = emit_spair(st, 0)
                        for kp in range(np_):
                            spair, di0, off0, off1 = pend
                            kt0 = 2 * kp
                            at = p2a.tile([P, 2, RB], BF16, tag="at")
                            if di0 >= 0:
                                # both halves diagonal: one paired mask-add +
                                # one paired exp over [off0:].  Half 1's
                                # [off0:off1) is mask=NEG -> exp ~ 0.
                                msk = p2t.tile([P, 2, RB], F32, tag="msk")
                                nc.vector.tensor_tensor(
                                    msk[:, :, off0:], spair[:, :, off0:],
                                    mask_t[:, di0 : di0 + 2, off0:], ADD)
                                nc.scalar.activation(
                                    at[:, :, off0:], msk[:, :, off0:],
                                    EXP, scale=SCALE)
                            else:
                                nc.scalar.activation(
                                    at[:, :, :], spair[:, :, :],
                                    EXP, scale=SCALE)
                            # prefetch next kt-pair's scores before the
                            # exp-dependent o-matmuls
                            if kp + 1 < np_:
                                pend = emit_spair(st, kp + 1)
                            for i in (0, 1):
                                kt = kt0 + i
                                off = off0 if i == 0 else off1
                                nc.tensor.matmul(
                                    o_ps[:, off:], v_rm[:, kt],
                                    at[:, i, off:],
                                    start=(kt == 0), stop=(kt == nkt - 1))
                                la = laccE if i == 0 else laccO
                                if kp == 0:
                                    nc.vector.tensor_copy(la[:], at[:, i])
                                else:
                                    nc.vector.tensor_tensor(
                                        la[:, off0:], la[:, off0:],
                                        at[:, i, off0:], ADD)
                        nc.vector.tensor_tensor(laccE[:], laccE[:], laccO[:],
                                                ADD)
                        l_ps = p2ps_l.tile([P, RB], F32, tag="lps")
                        nc.tensor.matmul(l_ps[:], ones_t[:], laccE[:],
                                         start=True, stop=True)
                        rl = p2t.tile([P, RB], F32, tag="rl")
                        nc.vector.reciprocal_approx_fast(rl[:], l_ps[:])
                        ot = p2t.tile([P, RB], BF16, tag="ot")
                        nc.vector.tensor_tensor(ot[:], o_ps[:], rl[:], MULT)
                        j = b * QBB + qb
                        nc.sync.dma_start(a2a_in[h][j, :, :], ot[:])
                        st["qb"] += 1

                    # Two batch-streams per head slot, interleaved at q-block
                    # granularity so each stream's exp/finalize latencies are
                    # covered by the other's matmul work.
                    for h in range(HPC):
                        streams = []
                        for b in range(B):
                            streams.append({
                                "h": h, "b": b, "qb": 0,
                                "kT": qkv_sb[:, 2 + h, b * T : (b + 1) * T],
                                "qTs": None,
                                "vT": qkv_sb[:, 4 + h, b * T : (b + 1) * T],
                                "v_rm": p2kv.tile([P, KTB, P], BF16,
                                                  tag="v_rm",
                                                  name=f"vrm{h}{b}"),
                            })
                        for qb in range(QBB):
                            for st in streams:
                                st["qTs"] = qkv_sb[
                                    :, st["h"],
                                    st["b"] * T + qb * RB : st["b"] * T + (qb + 1) * RB]
                                emit_qb(st)
                        nc.gpsimd.collective_compute(
                            "AllToAll", mybir.AluOpType.bypass,
                            ins=[a2a_in[h][:, :, :]], outs=[a2a_out[h][:, :, :]],
                            replica_groups=[list(range(NCORES))])
                        if h == 0:
                            # prefetch phase-4A inputs during h1 attention
                            nc.sync.dma_start(wp_pre[:],
                                              wproj[:, 0:4, 0:8, :])
                            for j in range(NCORES):
                                nc.sync.dma_start(otf_A[:, j],
                                                  a2a_out[0][j, :, :])

            # ============ Phase 4: proj + residual (even/odd head halves) ======
            with tc.tile_pool(name="p4o", bufs=1) as p4o:
                x2n = p4o.tile([P, DT, RB], BF16)
                x2 = p4o.tile([P, DT, RB], F32)
                with (
                    tc.tile_pool(name="p45", bufs=1) as p45,
                    tc.tile_pool(name="p4w", bufs=4) as p4w,
                    tc.tile_pool(name="p4t", bufs=2) as p4t,
                    tc.tile_pool(name="p4psA", bufs=2, space="PSUM") as p4psA,
                    tc.tile_pool(name="p4psB", bufs=2, space="PSUM") as p4psB,
                    tc.tile_pool(name="p45ps_ss", bufs=1, space="PSUM") as p45ps_ss,
                ):
                    # xres on the sync queue: it fires at attention end,
                    # concurrent with proj-A compute, and keeps the scalar
                    # queue free for the proj weight stream.  otf_B comes
                    # after it on sync but is gated on the h1 collective
                    # anyway.
                    xr = p45.tile([P, DT, RB], F32)
                    for dt in range(DT):
                        nc.sync.dma_start(xr[:, dt], xres[dt, :, :])
                    # phase A reads the prefetched otf_A/wp_pre tiles
                    # (host reordered wproj dt axis to [evens, odds])
                    for do in range(DT):
                        if do < 4:
                            wpA = wp_pre[:, do]
                        else:
                            wpAt = p4w.tile([P, 8, P], BF16, tag="wpA")
                            nc.scalar.dma_start(wpAt[:], wproj[:, do, 0:8, :])
                            wpA = wpAt[:]
                        psA = p4psA.tile([P, RB], F32, tag="ppsA")
                        for i in range(8):
                            nc.tensor.matmul(psA[:], wpA[:, i], otf_A[:, i],
                                             start=(i == 0), stop=(i == 7))
                        nc.vector.tensor_tensor(x2[:, do], psA[:], xr[:, do], ADD)
                    # phase B: odd (h1) tiles
                    otf_B = p45.tile([P, NCORES, RB], BF16)
                    for j in range(NCORES):
                        nc.sync.dma_start(otf_B[:, j], a2a_out[1][j, :, :])
                    for do in range(DT):
                        wpB = p4w.tile([P, 8, P], BF16, tag="wpB")
                        nc.scalar.dma_start(wpB[:], wproj[:, do, 8:16, :])
                        psB = p4psB.tile([P, RB], F32, tag="ppsB")
                        for i in range(8):
                            nc.tensor.matmul(psB[:], wpB[:, i], otf_B[:, i],
                                             start=(i == 0), stop=(i == 7))
                        nc.vector.tensor_tensor(x2[:, do], x2[:, do], psB[:], ADD)

                    # ---- norm2 ----
                    sacc2 = p4t.tile([P, RB], F32, tag="sacc2")
                    first = None
                    for dt in range(DT):
                        sq = p4t.tile([P, RB], F32, tag="sq2")
                        nc.scalar.activation(sq[:], x2[:, dt], SQUARE)
                        if dt == 0:
                            first = sq
                        elif dt == 1:
                            nc.vector.tensor_tensor(sacc2[:], first[:], sq[:], ADD)
                        else:
                            nc.vector.tensor_tensor(sacc2[:], sacc2[:], sq[:], ADD)
                    sacc2_bf = p4t.tile([P, RB], BF16, tag="sacc2bf")
                    nc.vector.tensor_copy(sacc2_bf[:], sacc2[:])
                    ss2 = p45ps_ss.tile([P, RB], F32, tag="ss2")
                    nc.tensor.matmul(ss2[:], ones_t[:], sacc2_bf[:],
                                     start=True, stop=True)
                    ms2 = p4t.tile([P, RB], F32, tag="ms2")
                    nc.vector.tensor_scalar(ms2[:], ss2[:], 1.0 / D, EPS, MULT, ADD)
                    nc.scalar.sqrt(ms2[:], ms2[:])
                    rms2 = p4t.tile([P, RB], F32, tag="rms2")
                    nc.vector.reciprocal_approx_fast(rms2[:], ms2[:])
                    for dt in range(DT):
                        nc.vector.tensor_tensor(x2n[:, dt], x2[:, dt], rms2[:], MULT)

                # ============ Phase 6: SwiGLU MLP (fg quarters) ============
                quarters = [(0, 11), (11, 22), (22, 33), (33, FGT)]
                out_acc = p4o.tile([P, DT, RB], F32)
                with (
                    tc.tile_pool(name="p6g", bufs=1) as p6g,
                    tc.tile_pool(name="p6w", bufs=4) as p6w,
                    tc.tile_pool(name="p6t", bufs=3) as p6t,
                    tc.tile_pool(name="p6ps_g", bufs=2, space="PSUM") as p6ps_g,
                    tc.tile_pool(name="p6ps_v", bufs=2, space="PSUM") as p6ps_v,
                    tc.tile_pool(name="p6ps_o", bufs=2, space="PSUM") as p6ps_o,
                ):
                    for qi, (fg0, fg1) in enumerate(quarters):
                        nq = fg1 - fg0
                        gt = p6g.tile([P, 11, RB], BF16, tag="gt")
                        for fi in range(nq):
                            fg = fg0 + fi
                            wg = p6w.tile([P, DT, P], BF16, tag="wg")
                            nc.scalar.dma_start(wg[:], wgate[:, fg, :, :])
                            wv = p6w.tile([P, DT, P], BF16, tag="wv")
                            nc.scalar.dma_start(wv[:], wval[:, fg, :, :])
                            g_ps = p6ps_g.tile([P, RB], F32, tag="gps")
                            for dt in range(DT):
                                nc.tensor.matmul(g_ps[:], wg[:, dt], x2n[:, dt],
                                                 start=(dt == 0), stop=(dt == DT - 1))
                            v_ps = p6ps_v.tile([P, RB], F32, tag="vps")
                            for dt in range(DT):
                                nc.tensor.matmul(v_ps[:], wv[:, dt], x2n[:, dt],
                                                 start=(dt == 0), stop=(dt == DT - 1))
                            sg = p6t.tile([P, RB], F32, tag="sg")
                            nc.scalar.activation(sg[:], g_ps[:], SILU)
                            nc.vector.tensor_tensor(gt[:, fi], sg[:], v_ps[:], MULT)
                        for do in range(DT):
                            wm = p6w.tile([P, 11, P], BF16, tag="wm")
                            nc.scalar.dma_start(
                                wm[:, :nq], wmlp[:, do, fg0:fg1, :])
                            o_ps = p6ps_o.tile([P, RB], F32, tag="ops6")
                            for fi in range(nq):
                                nc.tensor.matmul(o_ps[:], wm[:, fi], gt[:, fi],
                                                 start=(fi == 0), stop=(fi == nq - 1))
                            if qi == 0:
                                nc.vector.tensor_tensor(
                                    out_acc[:, do], o_ps[:], x2[:, do], ADD)
                            elif qi < len(quarters) - 1:
                                nc.vector.tensor_tensor(
                                    out_acc[:, do], o_ps[:], out_acc[:, do], ADD)
                            else:
                                fin = p6t.tile([P, RB], F32, tag="fin")
                                nc.vector.tensor_tensor(
                                    fin[:], o_ps[:], out_acc[:, do], ADD)
                                nc.sync.dma_start(outT[do, :, :], fin[:])

    nc.compile()
    return nc


def _rope_tables():
    inv_freq = 1.0 / (ROPE_BASE ** (np.arange(0, DH, 2, dtype=np.float32) / DH))
    t = np.arange(T, dtype=np.float32)
    freqs = np.outer(t, inv_freq)
    emb = np.repeat(freqs, 2, axis=-1)  # [T, DH]
    return np.cos(emb).astype(np.float32), np.sin(emb).astype(np.float32)


def _tile4(w, n_out_tiles, n_in_tiles):
    """[F_out, D_in] -> [P(p of d-tile), F_out/P tiles, D_in/P tiles, P(c of f-tile)].

    Element [p, f, dt, c] = w[f*P + c, dt*P + p].
    """
    Fo, Di = w.shape
    assert Fo == n_out_tiles * P and Di == n_in_tiles * P
    v = w.reshape(n_out_tiles, P, n_in_tiles, P)
    return np.ascontiguousarray(v.transpose(3, 0, 2, 1))


def _prepare_inputs(x, norm1_w, norm2_w, c_attn_w, c_proj_w, c_gate_w, c_val_w,
                    c_mlp_proj_w):
    xf = np.ascontiguousarray(x.reshape(R, D).T)  # [D, R] f32
    xf_bf = xf.astype(NPBF)
    cos, sin = _rope_tables()
    cosT = np.ascontiguousarray(cos.T).astype(NPBF)  # [DH, T]
    sinT = np.ascontiguousarray(sin.T).astype(NPBF)

    # rot-half signed permutation: (P @ q)[d] = -q[d+1] (d even), q[d-1] (d odd)
    rotP = np.zeros((P, P), np.float32)
    for d in range(0, P, 2):
        rotP[d, d + 1] = -1.0
        rotP[d + 1, d] = 1.0
    rotPT = np.ascontiguousarray(rotP.T).astype(NPBF)

    # additive causal masks for diagonal k-tiles, ST layout [k partition, q col]
    masks = np.zeros((P, QBB, RB), np.float32)
    for di in range(QBB):
        p_idx = np.arange(P)[:, None] + di * P
        c_idx = np.arange(RB)[None, :]
        masks[:, di, :] = np.where(p_idx <= c_idx, 0.0, NEG)
    masks = masks.astype(NPBF)

    ones_in = np.ones((P, P), NPBF)
    ident_in = np.eye(P, dtype=NPBF)

    w1 = norm1_w.astype(np.float32)
    w2 = norm2_w.astype(np.float32)
    attn_w = c_attn_w.astype(np.float32) * w1[None, :]     # fold norm1
    gate_w = c_gate_w.astype(np.float32) * w2[None, :]     # fold norm2
    val_w = c_val_w.astype(np.float32) * w2[None, :]

    gate_p = np.zeros((FGP, D), np.float32)
    gate_p[:FG] = gate_w
    val_p = np.zeros((FGP, D), np.float32)
    val_p[:FG] = val_w
    mlp_p = np.zeros((D, FGP), np.float32)
    mlp_p[:, :FG] = c_mlp_proj_w.astype(np.float32)

    wproj_t = _tile4(c_proj_w.astype(np.float32), DT, DT)
    # reorder contraction-tile axis to [even dts, odd dts] for the proj
    # phase-A/phase-B split (phase A = head-slot 0 tiles = even)
    dt_order = list(range(0, DT, 2)) + list(range(1, DT, 2))
    wproj_t = np.ascontiguousarray(wproj_t[:, :, dt_order, :]).astype(NPBF)
    wgate_t = _tile4(gate_p, FGT, DT).astype(NPBF)
    wval_t = _tile4(val_p, FGT, DT).astype(NPBF)
    # wmlp: lhsT [fg partition, dout col]: [p, do, fg, c] = mlp_p[do*P+c, fg*P+p]
    wmlp_t = np.ascontiguousarray(
        mlp_p.reshape(DT, P, FGT, P).transpose(3, 0, 2, 1)
    ).astype(NPBF)

    in_maps = []
    for i in range(NCORES):
        h0, h1 = 2 * i, 2 * i + 1
        rows = []
        for base in (0, D, 2 * D):  # q, k, v row groups of c_attn_w
            rows.extend(range(base + h0 * DH, base + h0 * DH + DH))
            rows.extend(range(base + h1 * DH, base + h1 * DH + DH))
        wsel = attn_w[rows, :]                       # [768, D]
        wqkv_t = _tile4(wsel, NF, DT).astype(NPBF)
        xres_i = np.ascontiguousarray(
            xf[:, i * RB : (i + 1) * RB].reshape(DT, P, RB)
        )
        in_maps.append({
            "xT": xf_bf,
            "xres": xres_i,
            "wqkv": wqkv_t,
            "wproj": wproj_t,
            "wgate": wgate_t,
            "wval": wval_t,
            "wmlp": wmlp_t,
            "cosT": cosT,
            "sinT": sinT,
            "rotPT": rotPT,
            "masks": masks,
            "ones_in": ones_in,
            "ident_in": ident_in,
        })
    return in_maps


_NC_CACHE = None


def _get_program():
    global _NC_CACHE
    if _NC_CACHE is None:
        _NC_CACHE = _build_program()
    return _NC_CACHE


def run(inputs, trace=False):
    """Returns (output [B,T,D], exec_time_ns or None)."""
    in_maps = _prepare_inputs(**inputs)
    nc = _get_program()
    res = run_bass_kernel_spmd(nc, in_maps, list(range(NCORES)), trace=trace)
    cols = []
    for i in range(NCORES):
        o = res.results[i]["outT"]          # [DT, P, RB]
        cols.append(o.reshape(D, RB))
    full_T = np.concatenate(cols, axis=1)   # [D, R]
    out = np.ascontiguousarray(full_T.T).reshape(B, T, D).astype(np.float32)
    return out, res.exec_time_ns


def kernel(**inputs) -> np.ndarray:
    out, _ = run(inputs, trace=False)
    return out

